# revision 3
# baseline (speedup 1.0000x reference)
"""DWT (db4-style, depthwise stride-2, reflect-pad) layer as a Trainium2
Bass/Tile kernel — bf16 datapath version.

Math: for input x[B, T, C] and 8-tap filters lo/hi the reference computes a
reflect-pad-7, stride-2, depthwise cross-correlation cropped by 3 per side:

    out[b, t', c]     = sum_k lo[k] * xe[b, 2 t' + k, c]
    out[b, t', C + c] = sum_k hi[k] * xe[b, 2 t' + k, c]

with xe[u] = x[u - 1] for u in [1, T+1), xe[0] = x[1], xe[T+1] = x[T-2]
(after the crop only one reflected element is needed per side), and
t' in [0, T/2 - 2).

Strategy (all per core; data-parallel over batch, Bl = 4 batches/core):
  - The HOST builds the exact overlapped device tile layout in bf16:
    x_dev[i, p, (h, w)] = xe[2*t0_i + 122*h + p][b, c], h in [0,4), so every
    device load is a plain contiguous [128 x 2 KB] DMA.  bf16 halves HBM
    traffic vs fp32 and runs the PE at 1 cycle/row instead of 4.
  - One stationary banded matrix W[p, s*61+q] = f_s[p - 2q] (s = lo|hi,
    p-2q in [0,8)) computes 61 outputs per 128-element window for both
    filters in a single matmul; the window index h lives purely in the rhs
    free dimension, so each super-tile is 2 matmuls of moving-free 512
    into a 2-bank PSUM tile [122, 1024] fp32.
  - One engine copy (alternating DVE / Activation) downcasts PSUM fp32 ->
    SBUF bf16 [122, 2 KB]; one DMA stores it (alternating Activation-HWDGE
    / GpSimd-SWDGE so no single sequencer serializes: each dma_start costs
    ~0.6 us on its issuing engine).
  - The last super-tile overlaps the previous one (t0 = T' - 244) so all 34
    super-tiles are identical; overlapping rows are recomputed bitwise
    identically, making the racing stores benign.
  - The host un-permutes [i, (s q), (h w)] -> [b, t', 2C] and upcasts.
"""

import numpy as np
import ml_dtypes

import concourse.bacc as bacc
import concourse.mybir as mybir
import concourse.tile as tile
from concourse.bass_utils import run_bass_kernel_spmd

F32 = mybir.dt.float32
BF16 = mybir.dt.bfloat16
BF16_NP = ml_dtypes.bfloat16

B, T, C = 32, 16384, 64
N_CORES = 8
BL = B // N_CORES           # 4 batches per core
NF = BL * C                 # 256 moving elements per time row
Q = 61                      # outputs per 128-wide window
H = 4                       # windows per super-tile
SUP = Q * H                 # 244 t' per super-tile
TP = T // 2 - 2             # 8190 output positions
NSUP = (TP + SUP - 1) // SUP  # 34 super-tiles (last one overlaps)
T0S = [SUP * i for i in range(NSUP - 1)] + [TP - SUP]


def _build_nc():
    """Single-core program: x_dev[NSUP,128,H*NF] bf16 -> o_dev[NSUP,122,H*NF] bf16."""
    nc = bacc.Bacc("TRN2", target_bir_lowering=False, debug=False)
    x_d = nc.dram_tensor("x", [NSUP, 128, H * NF], BF16, kind="ExternalInput")
    w_d = nc.dram_tensor("w", [128, 2 * Q], BF16, kind="ExternalInput")
    o_d = nc.dram_tensor("out", [NSUP, 2 * Q, H * NF], BF16, kind="ExternalOutput")

    with tile.TileContext(nc) as tc:
        with (
            tc.tile_pool(name="wpool", bufs=1) as wpool,
            tc.tile_pool(name="xin", bufs=6) as xpool,
            tc.tile_pool(name="oout", bufs=6) as opool,
            tc.tile_pool(name="ps", bufs=4, space="PSUM") as pspool,
        ):
            w_t = wpool.tile([128, 2 * Q], BF16)
            # scalar's HWDGE queue only spreads over 2 SDMA engines — keep
            # bulk data off it; sync's queue and SWDGE spread over all 16.
            nc.scalar.dma_start(out=w_t[:], in_=w_d[:])

            for i in range(NSUP):
                xt = xpool.tile([128, H * NF], BF16, tag="xt")
                nc.sync.dma_start(out=xt[:], in_=x_d[i])
                ps = pspool.tile([2 * Q, H * NF], F32, tag="ps")
                half = H * NF // 2  # 512 moving rows per matmul (one PSUM bank)
                for m in range(2):
                    nc.tensor.matmul(out=ps[:, half * m:half * (m + 1)],
                                     lhsT=w_t[:], rhs=xt[:, half * m:half * (m + 1)],
                                     start=True, stop=True)
                ot = opool.tile([2 * Q, H * NF], BF16, tag="ot")
                if i % 2 == 0:
                    nc.vector.tensor_copy(out=ot[:], in_=ps[:])
                    nc.sync.dma_start(out=o_d[i], in_=ot[:])
                else:
                    nc.scalar.copy(out=ot[:], in_=ps[:])
                    nc.gpsimd.dma_start(out=o_d[i], in_=ot[:])

    nc.compile()
    return nc


def _build_w(dec_lo: np.ndarray, dec_hi: np.ndarray) -> np.ndarray:
    """Banded stationary matrix [128, 2Q] bf16: cols [lo q=0..60 | hi q=0..60]."""
    w = np.zeros((128, 2 * Q), np.float32)
    for s, f in enumerate((np.asarray(dec_lo, np.float32),
                           np.asarray(dec_hi, np.float32))):
        for q in range(Q):
            w[2 * q:2 * q + 8, s * Q + q] = f
    return w.astype(BF16_NP)


def _prep_x(x: np.ndarray) -> list[np.ndarray]:
    """Per-core overlapped window layout [NSUP, 128, H*NF] bf16."""
    xb = x.astype(BF16_NP)
    t0 = np.asarray(T0S)
    # window start xe-index per (super, h, p)
    idx = (2 * t0[:, None, None] + 122 * np.arange(H)[None, :, None]
           + np.arange(128)[None, None, :])                    # [NSUP, H, 128]
    out = []
    for core in range(N_CORES):
        xc = np.ascontiguousarray(
            xb[core * BL:(core + 1) * BL].transpose(1, 0, 2))  # [T, BL, C]
        xe = np.concatenate([xc[1:2], xc, xc[T - 2:T - 1]], axis=0)  # [T+2, BL, C]
        xw = xe.reshape(T + 2, NF)[idx]                        # [NSUP, H, 128, NF]
        out.append(np.ascontiguousarray(
            xw.transpose(0, 2, 1, 3)).reshape(NSUP, 128, H * NF))
    return out


_NC_CACHE = {}


def _get_nc():
    if "nc" not in _NC_CACHE:
        _NC_CACHE["nc"] = _build_nc()
    return _NC_CACHE["nc"]


def kernel(x: np.ndarray, dec_lo: np.ndarray, dec_hi: np.ndarray) -> np.ndarray:
    x = np.asarray(x, np.float32)
    assert x.shape == (B, T, C), x.shape
    nc = _get_nc()
    w = _build_w(dec_lo, dec_hi)
    in_maps = [{"x": xc, "w": w} for xc in _prep_x(x)]
    res = run_bass_kernel_spmd(nc, in_maps, core_ids=list(range(N_CORES)))
    out = np.empty((B, TP, 2 * C), np.float32)
    for core in range(N_CORES):
        r = np.asarray(res.results[core]["out"]).reshape(NSUP, 2, Q, H, BL, C)
        # (i, s, q, h, b, c) -> [b, 244i + 61h + q, s, c]
        main = r[:NSUP - 1].transpose(4, 0, 3, 2, 1, 5).reshape(
            BL, (NSUP - 1) * SUP, 2 * C)
        last = r[NSUP - 1].transpose(3, 2, 1, 0, 4).reshape(BL, SUP, 2 * C)
        oc = out[core * BL:(core + 1) * BL]
        oc[:, :(NSUP - 1) * SUP] = main
        oc[:, TP - SUP:] = last
    return out


# revision 4
# speedup vs baseline: 1.2633x; 1.2633x over previous
"""DWT (db4-style, depthwise stride-2, reflect-pad) layer as a Trainium2
Bass/Tile kernel — bf16 datapath version.

Math: for input x[B, T, C] and 8-tap filters lo/hi the reference computes a
reflect-pad-7, stride-2, depthwise cross-correlation cropped by 3 per side:

    out[b, t', c]     = sum_k lo[k] * xe[b, 2 t' + k, c]
    out[b, t', C + c] = sum_k hi[k] * xe[b, 2 t' + k, c]

with xe[u] = x[u - 1] for u in [1, T+1), xe[0] = x[1], xe[T+1] = x[T-2]
(after the crop only one reflected element is needed per side), and
t' in [0, T/2 - 2).

Strategy (all per core; data-parallel over batch, Bl = 4 batches/core):
  - The HOST builds the exact overlapped device tile layout in bf16:
    x_dev[i, p, (h, w)] = xe[2*t0_i + 122*h + p][b, c], h in [0,4), so every
    device load is a plain contiguous [128 x 2 KB] DMA.  bf16 halves HBM
    traffic vs fp32 and runs the PE at 1 cycle/row instead of 4.
  - One stationary banded matrix W[p, s*61+q] = f_s[p - 2q] (s = lo|hi,
    p-2q in [0,8)) computes 61 outputs per 128-element window for both
    filters in a single matmul; the window index h lives purely in the rhs
    free dimension, so each super-tile is 2 matmuls of moving-free 512
    into a 2-bank PSUM tile [122, 1024] fp32.
  - One engine copy (alternating DVE / Activation) downcasts PSUM fp32 ->
    SBUF bf16 [122, 2 KB]; one DMA stores it (alternating Activation-HWDGE
    / GpSimd-SWDGE so no single sequencer serializes: each dma_start costs
    ~0.6 us on its issuing engine).
  - The last super-tile overlaps the previous one (t0 = T' - 244) so all 34
    super-tiles are identical; overlapping rows are recomputed bitwise
    identically, making the racing stores benign.
  - The host un-permutes [i, (s q), (h w)] -> [b, t', 2C] and upcasts.
"""

import numpy as np
import ml_dtypes

import concourse.bacc as bacc
import concourse.mybir as mybir
import concourse.tile as tile
from concourse.bass_utils import run_bass_kernel_spmd

F32 = mybir.dt.float32
BF16 = mybir.dt.bfloat16
BF16_NP = ml_dtypes.bfloat16

B, T, C = 32, 16384, 64
N_CORES = 8
BL = B // N_CORES           # 4 batches per core
NF = BL * C                 # 256 moving elements per time row
Q = 61                      # outputs per 128-wide window
H = 4                       # windows per super-tile
SUP = Q * H                 # 244 t' per super-tile
TP = T // 2 - 2             # 8190 output positions
NSUP = (TP + SUP - 1) // SUP  # 34 super-tiles (last one overlaps)
T0S = [SUP * i for i in range(NSUP - 1)] + [TP - SUP]


def _build_nc():
    """Single-core program: x_dev[NSUP,128,H*NF] bf16 -> o_dev[NSUP,122,H*NF] bf16."""
    nc = bacc.Bacc("TRN2", target_bir_lowering=False, debug=False)
    x_d = nc.dram_tensor("x", [NSUP, 128, H * NF], BF16, kind="ExternalInput")
    w_d = nc.dram_tensor("w", [128, 2 * Q], BF16, kind="ExternalInput")
    o_d = nc.dram_tensor("out", [NSUP, 2 * Q, H * NF], BF16, kind="ExternalOutput")

    with tile.TileContext(nc) as tc:
        with (
            tc.tile_pool(name="wpool", bufs=1) as wpool,
            tc.tile_pool(name="xin", bufs=6) as xpool,
            tc.tile_pool(name="oout", bufs=6) as opool,
            tc.tile_pool(name="ps", bufs=4, space="PSUM") as pspool,
        ):
            w_t = wpool.tile([128, 2 * Q], BF16)
            # scalar's HWDGE queue only spreads over 2 SDMA engines — keep
            # bulk data off it; sync's queue and SWDGE spread over all 16.
            nc.scalar.dma_start(out=w_t[:], in_=w_d[:])

            for i in range(NSUP):
                xt = xpool.tile([128, H * NF], BF16, tag="xt")
                nc.sync.dma_start(out=xt[:], in_=x_d[i])
                ps = pspool.tile([2 * Q, H * NF], F32, tag="ps")
                half = H * NF // 2  # 512 moving rows per matmul (one PSUM bank)
                for m in range(2):
                    nc.tensor.matmul(out=ps[:, half * m:half * (m + 1)],
                                     lhsT=w_t[:], rhs=xt[:, half * m:half * (m + 1)],
                                     start=True, stop=True)
                ot = opool.tile([2 * Q, H * NF], BF16, tag="ot")
                if i % 2 == 0:
                    nc.vector.tensor_copy(out=ot[:], in_=ps[:])
                else:
                    nc.scalar.copy(out=ot[:], in_=ps[:])
                # HWDGE DRAM *writes* ride only 2 of the 16 SDMA engines;
                # SWDGE spreads all 16 (and aggregates 4KB packets), so all
                # bulk stores go through gpsimd.
                nc.gpsimd.dma_start(out=o_d[i], in_=ot[:])

    nc.compile()
    return nc


def _build_w(dec_lo: np.ndarray, dec_hi: np.ndarray) -> np.ndarray:
    """Banded stationary matrix [128, 2Q] bf16: cols [lo q=0..60 | hi q=0..60]."""
    w = np.zeros((128, 2 * Q), np.float32)
    for s, f in enumerate((np.asarray(dec_lo, np.float32),
                           np.asarray(dec_hi, np.float32))):
        for q in range(Q):
            w[2 * q:2 * q + 8, s * Q + q] = f
    return w.astype(BF16_NP)


def _prep_x(x: np.ndarray) -> list[np.ndarray]:
    """Per-core overlapped window layout [NSUP, 128, H*NF] bf16."""
    xb = x.astype(BF16_NP)
    t0 = np.asarray(T0S)
    # window start xe-index per (super, h, p)
    idx = (2 * t0[:, None, None] + 122 * np.arange(H)[None, :, None]
           + np.arange(128)[None, None, :])                    # [NSUP, H, 128]
    out = []
    for core in range(N_CORES):
        xc = np.ascontiguousarray(
            xb[core * BL:(core + 1) * BL].transpose(1, 0, 2))  # [T, BL, C]
        xe = np.concatenate([xc[1:2], xc, xc[T - 2:T - 1]], axis=0)  # [T+2, BL, C]
        xw = xe.reshape(T + 2, NF)[idx]                        # [NSUP, H, 128, NF]
        out.append(np.ascontiguousarray(
            xw.transpose(0, 2, 1, 3)).reshape(NSUP, 128, H * NF))
    return out


_NC_CACHE = {}


def _get_nc():
    if "nc" not in _NC_CACHE:
        _NC_CACHE["nc"] = _build_nc()
    return _NC_CACHE["nc"]


def kernel(x: np.ndarray, dec_lo: np.ndarray, dec_hi: np.ndarray) -> np.ndarray:
    x = np.asarray(x, np.float32)
    assert x.shape == (B, T, C), x.shape
    nc = _get_nc()
    w = _build_w(dec_lo, dec_hi)
    in_maps = [{"x": xc, "w": w} for xc in _prep_x(x)]
    res = run_bass_kernel_spmd(nc, in_maps, core_ids=list(range(N_CORES)))
    out = np.empty((B, TP, 2 * C), np.float32)
    for core in range(N_CORES):
        r = np.asarray(res.results[core]["out"]).reshape(NSUP, 2, Q, H, BL, C)
        # (i, s, q, h, b, c) -> [b, 244i + 61h + q, s, c]
        main = r[:NSUP - 1].transpose(4, 0, 3, 2, 1, 5).reshape(
            BL, (NSUP - 1) * SUP, 2 * C)
        last = r[NSUP - 1].transpose(3, 2, 1, 0, 4).reshape(BL, SUP, 2 * C)
        oc = out[core * BL:(core + 1) * BL]
        oc[:, :(NSUP - 1) * SUP] = main
        oc[:, TP - SUP:] = last
    return out


# revision 7
# speedup vs baseline: 1.3380x; 1.0591x over previous
"""DWT (db4-style, depthwise stride-2, reflect-pad) layer as a Trainium2
Bass/Tile kernel — bf16 datapath version.

Math: for input x[B, T, C] and 8-tap filters lo/hi the reference computes a
reflect-pad-7, stride-2, depthwise cross-correlation cropped by 3 per side:

    out[b, t', c]     = sum_k lo[k] * xe[b, 2 t' + k, c]
    out[b, t', C + c] = sum_k hi[k] * xe[b, 2 t' + k, c]

with xe[u] = x[u - 1] for u in [1, T+1), xe[0] = x[1], xe[T+1] = x[T-2]
(after the crop only one reflected element is needed per side), and
t' in [0, T/2 - 2).

Strategy (all per core; data-parallel over batch, Bl = 4 batches/core):
  - The HOST builds the exact overlapped device tile layout in bf16:
    x_dev[i, p, (h, w)] = xe[2*t0_i + 122*h + p][b, c], h in [0,4), so every
    device load is a plain contiguous [128 x 2 KB] DMA.  bf16 halves HBM
    traffic vs fp32 and runs the PE at 1 cycle/row instead of 4.
  - One stationary banded matrix W[p, s*61+q] = f_s[p - 2q] (s = lo|hi,
    p-2q in [0,8)) computes 61 outputs per 128-element window for both
    filters in a single matmul; the window index h lives purely in the rhs
    free dimension, so each super-tile is 2 matmuls of moving-free 512
    into a 2-bank PSUM tile [122, 1024] fp32.
  - One engine copy (alternating DVE / Activation) downcasts PSUM fp32 ->
    SBUF bf16 [122, 2 KB]; one DMA stores it (alternating Activation-HWDGE
    / GpSimd-SWDGE so no single sequencer serializes: each dma_start costs
    ~0.6 us on its issuing engine).
  - The last super-tile overlaps the previous one (t0 = T' - 244) so all 34
    super-tiles are identical; overlapping rows are recomputed bitwise
    identically, making the racing stores benign.
  - The host un-permutes [i, (s q), (h w)] -> [b, t', 2C] and upcasts.
"""

import numpy as np
import ml_dtypes

import concourse.bacc as bacc
import concourse.mybir as mybir
import concourse.tile as tile
from concourse.bass_utils import run_bass_kernel_spmd

F32 = mybir.dt.float32
BF16 = mybir.dt.bfloat16
BF16_NP = ml_dtypes.bfloat16

B, T, C = 32, 16384, 64
N_CORES = 8
BL = B // N_CORES           # 4 batches per core
NF = BL * C                 # 256 moving elements per time row
Q = 61                      # outputs per 128-wide window
H = 8                       # windows per super-tile
SUP = Q * H                 # 488 t' per super-tile
TP = T // 2 - 2             # 8190 output positions
NSUP = (TP + SUP - 1) // SUP  # 17 super-tiles (last one overlaps)
T0S = [SUP * i for i in range(NSUP - 1)] + [TP - SUP]


def _build_nc():
    """Single-core program: x_dev[NSUP,128,H*NF] bf16 -> o_dev[NSUP,122,H*NF] bf16."""
    nc = bacc.Bacc("TRN2", target_bir_lowering=False, debug=False)
    x_d = nc.dram_tensor("x", [NSUP, 128, H * NF], BF16, kind="ExternalInput")
    w_d = nc.dram_tensor("w", [128, 2 * Q], BF16, kind="ExternalInput")
    o_d = nc.dram_tensor("out", [NSUP, 2 * Q, H * NF], BF16, kind="ExternalOutput")

    with tile.TileContext(nc) as tc:
        with (
            tc.tile_pool(name="wpool", bufs=1) as wpool,
            tc.tile_pool(name="xin", bufs=4) as xpool,
            tc.tile_pool(name="oout", bufs=4) as opool,
            tc.tile_pool(name="ps", bufs=4, space="PSUM") as pspool,
        ):
            w_t = wpool.tile([128, 2 * Q], BF16)
            # scalar's HWDGE queue only spreads over 2 SDMA engines — keep
            # bulk data off it; sync's queue and SWDGE spread over all 16.
            nc.scalar.dma_start(out=w_t[:], in_=w_d[:])

            half = H * NF // 2          # 1024 f32 = one 2-bank PSUM tile
            for i in range(NSUP):
                xt = xpool.tile([128, H * NF], BF16, tag="xt")
                nc.sync.dma_start(out=xt[:], in_=x_d[i])
                ot = opool.tile([2 * Q, H * NF], BF16, tag="ot")
                for m in range(2):
                    ps = pspool.tile([2 * Q, half], F32, tag="ps")
                    for j in range(2):  # 512 moving rows per matmul (one bank)
                        lo = (half // 2) * j
                        nc.tensor.matmul(out=ps[:, lo:lo + half // 2],
                                         lhsT=w_t[:],
                                         rhs=xt[:, half * m + lo:half * m + lo + half // 2],
                                         start=True, stop=True)
                    # both copy engines work the same super in parallel
                    eng = nc.vector.tensor_copy if m == 0 else nc.scalar.copy
                    eng(out=ot[:, half * m:half * (m + 1)], in_=ps[:])
                # HWDGE DRAM *writes* ride only 2 of the 16 SDMA engines;
                # SWDGE spreads all 16 (and aggregates 4KB packets), so all
                # bulk stores go through gpsimd.
                nc.gpsimd.dma_start(out=o_d[i], in_=ot[:])

    nc.compile()
    return nc


def _build_w(dec_lo: np.ndarray, dec_hi: np.ndarray) -> np.ndarray:
    """Banded stationary matrix [128, 2Q] bf16: cols [lo q=0..60 | hi q=0..60]."""
    w = np.zeros((128, 2 * Q), np.float32)
    for s, f in enumerate((np.asarray(dec_lo, np.float32),
                           np.asarray(dec_hi, np.float32))):
        for q in range(Q):
            w[2 * q:2 * q + 8, s * Q + q] = f
    return w.astype(BF16_NP)


def _prep_x(x: np.ndarray) -> list[np.ndarray]:
    """Per-core overlapped window layout [NSUP, 128, H*NF] bf16."""
    xb = x.astype(BF16_NP)
    t0 = np.asarray(T0S)
    # window start xe-index per (super, h, p)
    idx = (2 * t0[:, None, None] + 122 * np.arange(H)[None, :, None]
           + np.arange(128)[None, None, :])                    # [NSUP, H, 128]
    out = []
    for core in range(N_CORES):
        xc = np.ascontiguousarray(
            xb[core * BL:(core + 1) * BL].transpose(1, 0, 2))  # [T, BL, C]
        xe = np.concatenate([xc[1:2], xc, xc[T - 2:T - 1]], axis=0)  # [T+2, BL, C]
        xw = xe.reshape(T + 2, NF)[idx]                        # [NSUP, H, 128, NF]
        out.append(np.ascontiguousarray(
            xw.transpose(0, 2, 1, 3)).reshape(NSUP, 128, H * NF))
    return out


_NC_CACHE = {}


def _get_nc():
    if "nc" not in _NC_CACHE:
        _NC_CACHE["nc"] = _build_nc()
    return _NC_CACHE["nc"]


def kernel(x: np.ndarray, dec_lo: np.ndarray, dec_hi: np.ndarray) -> np.ndarray:
    x = np.asarray(x, np.float32)
    assert x.shape == (B, T, C), x.shape
    nc = _get_nc()
    w = _build_w(dec_lo, dec_hi)
    in_maps = [{"x": xc, "w": w} for xc in _prep_x(x)]
    res = run_bass_kernel_spmd(nc, in_maps, core_ids=list(range(N_CORES)))
    out = np.empty((B, TP, 2 * C), np.float32)
    for core in range(N_CORES):
        r = np.asarray(res.results[core]["out"]).reshape(NSUP, 2, Q, H, BL, C)
        # (i, s, q, h, b, c) -> [b, 244i + 61h + q, s, c]
        main = r[:NSUP - 1].transpose(4, 0, 3, 2, 1, 5).reshape(
            BL, (NSUP - 1) * SUP, 2 * C)
        last = r[NSUP - 1].transpose(3, 2, 1, 0, 4).reshape(BL, SUP, 2 * C)
        oc = out[core * BL:(core + 1) * BL]
        oc[:, :(NSUP - 1) * SUP] = main
        oc[:, TP - SUP:] = last
    return out


# revision 9
# speedup vs baseline: 1.9614x; 1.4660x over previous
"""DWT (db4-style, depthwise stride-2, reflect-pad) layer as a Trainium2
Bass/Tile kernel — bf16 datapath version.

Math: for input x[B, T, C] and 8-tap filters lo/hi the reference computes a
reflect-pad-7, stride-2, depthwise cross-correlation cropped by 3 per side:

    out[b, t', c]     = sum_k lo[k] * xe[b, 2 t' + k, c]
    out[b, t', C + c] = sum_k hi[k] * xe[b, 2 t' + k, c]

with xe[u] = x[u - 1] for u in [1, T+1), xe[0] = x[1], xe[T+1] = x[T-2]
(after the crop only one reflected element is needed per side), and
t' in [0, T/2 - 2).

Strategy (all per core; data-parallel over batch, Bl = 4 batches/core):
  - The HOST builds the exact overlapped device tile layout in bf16:
    x_dev[i, p, (h, w)] = xe[2*t0_i + 122*h + p][b, c], h in [0,4), so every
    device load is a plain contiguous [128 x 2 KB] DMA.  bf16 halves HBM
    traffic vs fp32 and runs the PE at 1 cycle/row instead of 4.
  - One stationary banded matrix W[p, s*61+q] = f_s[p - 2q] (s = lo|hi,
    p-2q in [0,8)) computes 61 outputs per 128-element window for both
    filters in a single matmul; the window index h lives purely in the rhs
    free dimension, so each super-tile is 2 matmuls of moving-free 512
    into a 2-bank PSUM tile [122, 1024] fp32.
  - One engine copy (alternating DVE / Activation) downcasts PSUM fp32 ->
    SBUF bf16 [122, 2 KB]; one DMA stores it (alternating Activation-HWDGE
    / GpSimd-SWDGE so no single sequencer serializes: each dma_start costs
    ~0.6 us on its issuing engine).
  - The last super-tile overlaps the previous one (t0 = T' - 244) so all 34
    super-tiles are identical; overlapping rows are recomputed bitwise
    identically, making the racing stores benign.
  - The host un-permutes [i, (s q), (h w)] -> [b, t', 2C] and upcasts.
"""

import numpy as np
import ml_dtypes

import concourse.bacc as bacc
import concourse.mybir as mybir
import concourse.tile as tile
from concourse.bass_utils import run_bass_kernel_spmd

F32 = mybir.dt.float32
BF16 = mybir.dt.bfloat16
BF16_NP = ml_dtypes.bfloat16

B, T, C = 32, 16384, 64
N_CORES = 8
BL = B // N_CORES           # 4 batches per core
NF = BL * C                 # 256 moving elements per time row
Q = 61                      # outputs per 128-wide window
H = 8                       # windows per super-tile
SUP = Q * H                 # 488 t' per super-tile
TP = T // 2 - 2             # 8190 output positions
NSUP = (TP + SUP - 1) // SUP  # 17 super-tiles (last one overlaps)
T0S = [SUP * i for i in range(NSUP - 1)] + [TP - SUP]


def _build_nc():
    """Single-core program: x_dev[NSUP,128,H*NF] bf16 -> o_dev[NSUP,122,H*NF] bf16."""
    nc = bacc.Bacc("TRN2", target_bir_lowering=False, debug=False)
    x_d = nc.dram_tensor("x", [NSUP, 128, H * NF], BF16, kind="ExternalInput")
    w_d = nc.dram_tensor("w", [128, 2 * Q], BF16, kind="ExternalInput")
    o_d = nc.dram_tensor("out", [NSUP, 2 * Q, H * NF], BF16, kind="ExternalOutput")

    with tile.TileContext(nc) as tc:
        with (
            tc.tile_pool(name="wpool", bufs=1) as wpool,
            tc.tile_pool(name="xin", bufs=6) as xpool,
            tc.tile_pool(name="oout", bufs=6) as opool,
            tc.tile_pool(name="ps", bufs=4, space="PSUM") as pspool,
        ):
            w_t = wpool.tile([128, 2 * Q], BF16)
            # scalar's HWDGE queue only spreads over 2 SDMA engines — keep
            # bulk data off it; sync's queue and SWDGE spread over all 16.
            nc.scalar.dma_start(out=w_t[:], in_=w_d[:])

            half = H * NF // 2          # 1024 f32 = one 2-bank PSUM tile
            for i in range(NSUP):
                xt = xpool.tile([128, H * NF], BF16, tag="xt")
                nc.sync.dma_start(out=xt[:], in_=x_d[i])
                ot = opool.tile([2 * Q, H * NF], BF16, tag="ot")
                for m in range(2):
                    ps = pspool.tile([2 * Q, half], F32, tag="ps")
                    for j in range(2):  # 512 moving rows per matmul (one bank)
                        lo = (half // 2) * j
                        nc.tensor.matmul(out=ps[:, lo:lo + half // 2],
                                         lhsT=w_t[:],
                                         rhs=xt[:, half * m + lo:half * m + lo + half // 2],
                                         start=True, stop=True)
                    # both copy engines work the same super in parallel
                    eng = nc.vector.tensor_copy if m == 0 else nc.scalar.copy
                    eng(out=ot[:, half * m:half * (m + 1)], in_=ps[:])
                # Stores are the bottleneck: SWDGE spreads all 16 SDMA engines
                # but its 512B-subdescriptor ring drains at only ~105 GB/s;
                # HWDGE DRAM writes ride just 2 engines (~45 GB/s).  Split
                # each store ~79/21 so both paths finish together.
                psplit = 96
                nc.gpsimd.dma_start(out=o_d[i, :psplit], in_=ot[:psplit])
                nc.sync.dma_start(out=o_d[i, psplit:], in_=ot[psplit:2 * Q])

    nc.compile()
    return nc


def _build_w(dec_lo: np.ndarray, dec_hi: np.ndarray) -> np.ndarray:
    """Banded stationary matrix [128, 2Q] bf16: cols [lo q=0..60 | hi q=0..60]."""
    w = np.zeros((128, 2 * Q), np.float32)
    for s, f in enumerate((np.asarray(dec_lo, np.float32),
                           np.asarray(dec_hi, np.float32))):
        for q in range(Q):
            w[2 * q:2 * q + 8, s * Q + q] = f
    return w.astype(BF16_NP)


def _prep_x(x: np.ndarray) -> list[np.ndarray]:
    """Per-core overlapped window layout [NSUP, 128, H*NF] bf16."""
    xb = x.astype(BF16_NP)
    t0 = np.asarray(T0S)
    # window start xe-index per (super, h, p)
    idx = (2 * t0[:, None, None] + 122 * np.arange(H)[None, :, None]
           + np.arange(128)[None, None, :])                    # [NSUP, H, 128]
    out = []
    for core in range(N_CORES):
        xc = np.ascontiguousarray(
            xb[core * BL:(core + 1) * BL].transpose(1, 0, 2))  # [T, BL, C]
        xe = np.concatenate([xc[1:2], xc, xc[T - 2:T - 1]], axis=0)  # [T+2, BL, C]
        xw = xe.reshape(T + 2, NF)[idx]                        # [NSUP, H, 128, NF]
        out.append(np.ascontiguousarray(
            xw.transpose(0, 2, 1, 3)).reshape(NSUP, 128, H * NF))
    return out


_NC_CACHE = {}


def _get_nc():
    if "nc" not in _NC_CACHE:
        _NC_CACHE["nc"] = _build_nc()
    return _NC_CACHE["nc"]


def kernel(x: np.ndarray, dec_lo: np.ndarray, dec_hi: np.ndarray) -> np.ndarray:
    x = np.asarray(x, np.float32)
    assert x.shape == (B, T, C), x.shape
    nc = _get_nc()
    w = _build_w(dec_lo, dec_hi)
    in_maps = [{"x": xc, "w": w} for xc in _prep_x(x)]
    res = run_bass_kernel_spmd(nc, in_maps, core_ids=list(range(N_CORES)))
    out = np.empty((B, TP, 2 * C), np.float32)
    for core in range(N_CORES):
        r = np.asarray(res.results[core]["out"]).reshape(NSUP, 2, Q, H, BL, C)
        # (i, s, q, h, b, c) -> [b, 244i + 61h + q, s, c]
        main = r[:NSUP - 1].transpose(4, 0, 3, 2, 1, 5).reshape(
            BL, (NSUP - 1) * SUP, 2 * C)
        last = r[NSUP - 1].transpose(3, 2, 1, 0, 4).reshape(BL, SUP, 2 * C)
        oc = out[core * BL:(core + 1) * BL]
        oc[:, :(NSUP - 1) * SUP] = main
        oc[:, TP - SUP:] = last
    return out
